# revision 4
# baseline (speedup 1.0000x reference)
"""Trainium2 Bass kernel for nn_BartAttention_66786741453241 (8 NeuronCores).

Reference (bugs preserved): no softmax — raw attention scores are used for the
AV matmul, and q is scaled by dh**-0.5 with scores further divided by sqrt(dh),
net 1/dh. The whole computation is therefore LINEAR in V, so we reassociate
    (Q K^T / 64) V  ==  Q (K^T V) / 64
which collapses the [T,T] score matrices into per-head [64,64] K^T V matrices
(~32x fewer attention FLOPs, exact in infinite precision).

Sharding: tokens. 4096 tokens (B*T) split across 8 cores, 512 each; cores 0-3
hold batch 0, cores 4-7 batch 1. Per core:
  - q/k/v projections for its own 512 tokens (full heads),
  - partial K^T V over its own tokens (per head),
  - grouped AllReduce (ranks [0-3] and [4-7]) of the 16x[64,64] partial KTVs
    (256 KB, f32) -> full-batch KTV,
  - O^T = KTV^T-free form: per head OT_h = KTV_h^T-less matmul lhsT=KTV_h,
    rhs = qT_h, then out^T = Wo^T-proj of O^T (+bo), DMA'd out as [E, 512].
Host side: weights pre-transposed to [e_in, e_out] bf16, hs pre-transposed to
[E, tokens] bf16, biases pre-scaled; output chunks transposed+concatenated.
All matmuls run in bf16 (fp32 PSUM accumulate): measured end-to-end relative
error vs the f32 reference ~4.5e-3.
"""

import os
import sys
import types

import numpy as np
import ml_dtypes

import concourse.bass as bass
import concourse.bacc as bacc
import concourse.mybir as mybir
import concourse.tile as tile
from concourse.bass_utils import run_bass_kernel_spmd

BF16 = mybir.dt.bfloat16
F32 = mybir.dt.float32
NPBF16 = ml_dtypes.bfloat16

E = 1024        # embed dim
H = 16          # heads
DH = 64         # head dim
B, T = 2, 2048
NTOK = B * T    # 4096
NC = 8          # cores
TPC = NTOK // NC  # 512 tokens per core
P = 128
KC = E // P     # 8 contraction chunks
Ident = mybir.ActivationFunctionType.Identity


def _install_axon_profile_hook():
    """Make trace=True usable under axon: register the NTFF hook that the
    staged antenv lacks, and neuter artifact upload (no bucket here). Safe
    no-op when pieces are missing."""
    try:
        import concourse.bass_utils as bu
        bu.upload_artifacts = lambda tmpdir: "local://" + tmpdir
    except Exception:
        pass
    if "antenv.axon_hooks" in sys.modules:
        return
    hook = None
    try:
        from trn_agent_boot.trn_boot import _ntff_profile_via_ctypes
        so = "/opt/axon/libaxon_pjrt.so"
        if os.path.exists(so):
            hook = _ntff_profile_via_ctypes(so)
    except Exception:
        hook = None
    mod = types.ModuleType("antenv.axon_hooks")
    mod.get_axon_ntff_profile_hook = lambda: hook
    mod.set_axon_ntff_profile_hook = lambda h: None
    sys.modules["antenv.axon_hooks"] = mod


def build():
    """Build + compile the per-core SPMD graph (identical on all 8 cores)."""
    nc = bacc.Bacc("TRN2", target_bir_lowering=False, debug=False, num_devices=NC)

    hsT = nc.dram_tensor("hsT", [E, TPC], BF16, kind="ExternalInput")
    wqt = nc.dram_tensor("wqt", [E, E], BF16, kind="ExternalInput")
    wkt = nc.dram_tensor("wkt", [E, E], BF16, kind="ExternalInput")
    wvt = nc.dram_tensor("wvt", [E, E], BF16, kind="ExternalInput")
    wot = nc.dram_tensor("wot", [E, E], BF16, kind="ExternalInput")
    bq64 = nc.dram_tensor("bq64", [E], F32, kind="ExternalInput")
    bo_d = nc.dram_tensor("bo", [E], F32, kind="ExternalInput")
    bkb = nc.dram_tensor("bkb", [P, E], F32, kind="ExternalInput")
    bvb = nc.dram_tensor("bvb", [P, E], F32, kind="ExternalInput")
    outT = nc.dram_tensor("outT", [E, TPC], F32, kind="ExternalOutput")

    with tile.TileContext(nc) as tc:
        with (
            tc.tile_pool(name="sb", bufs=1) as sb,
            tc.tile_pool(name="stg", bufs=3) as stg,
            tc.tile_pool(name="psA", bufs=4, space="PSUM") as psA,
            tc.tile_pool(name="psB", bufs=2, space="PSUM") as psB,
            tc.tile_pool(name="psC", bufs=2, space="PSUM") as psC,
            tc.tile_pool(name="dram", bufs=1, space="DRAM") as dram,
        ):
            # ---- phase 1 loads: hsT, k/v weights, k/v bias broadcast tiles
            hs_sb = []
            for c in range(KC):
                t_ = sb.tile([P, TPC], BF16, tag=f"hs{c}")
                nc.sync.dma_start(t_[:], hsT[c * P:(c + 1) * P, :])
                hs_sb.append(t_)
            wk_sb, wv_sb = [], []
            for c in range(KC):
                tk = sb.tile([P, E], BF16, tag=f"wk{c}")
                nc.sync.dma_start(tk[:], wkt[c * P:(c + 1) * P, :])
                wk_sb.append(tk)
                tv = sb.tile([P, E], BF16, tag=f"wv{c}")
                nc.sync.dma_start(tv[:], wvt[c * P:(c + 1) * P, :])
                wv_sb.append(tv)
            bkb_sb = sb.tile([P, E], F32, tag="bkb")
            nc.sync.dma_start(bkb_sb[:], bkb[:, :])
            bvb_sb = sb.tile([P, E], F32, tag="bvb")
            nc.sync.dma_start(bvb_sb[:], bvb[:, :])

            # ---- k, v projections (natural layout [tokens, e_out])
            TT = TPC // P  # 4 token chunks
            k_sb = [sb.tile([P, E], BF16, tag=f"k{tt}", name=f"k{tt}") for tt in range(TT)]
            v_sb = [sb.tile([P, E], BF16, tag=f"v{tt}", name=f"v{tt}") for tt in range(TT)]
            for dst, w_sb, bias_sb in ((k_sb, wk_sb, bkb_sb), (v_sb, wv_sb, bvb_sb)):
                for tt in range(TT):
                    for half in range(2):
                        ps = psA.tile([P, 512], F32, tag="psA")
                        for c in range(KC):
                            nc.tensor.matmul(
                                ps[:],
                                hs_sb[c][:, tt * P:(tt + 1) * P],
                                w_sb[c][:, half * 512:(half + 1) * 512],
                                start=(c == 0),
                                stop=(c == KC - 1),
                            )
                        nc.vector.tensor_add(
                            dst[tt][:, half * 512:(half + 1) * 512],
                            ps[:],
                            bias_sb[:, half * 512:(half + 1) * 512],
                        )

            # ---- per-head partial K^T V  -> staged [128, H/2*64] f32
            # head pairs share a column block: head 2j at partitions 0-63,
            # head 2j+1 at partitions 64-127 (so lhsT/rhs base partitions
            # match in the Q@KTV matmul below).
            ktv_stage = sb.tile([P, (H // 2) * DH], F32, tag="ktv_stage")
            for h in range(H):
                r0 = (h % 2) * DH
                j = h // 2
                ps = psB.tile([P, DH], F32, tag="psB")
                for tt in range(TT):
                    nc.tensor.matmul(
                        ps[r0:r0 + DH, :],
                        k_sb[tt][:, h * DH:(h + 1) * DH],
                        v_sb[tt][:, h * DH:(h + 1) * DH],
                        start=(tt == 0),
                        stop=(tt == TT - 1),
                    )
                nc.vector.tensor_copy(
                    ktv_stage[r0:r0 + DH, j * DH:(j + 1) * DH], ps[r0:r0 + DH, :]
                )

            # ---- grouped AllReduce of partial KTVs (within each batch)
            in_b = dram.tile([P, (H // 2) * DH], F32)
            out_b = dram.tile([P, (H // 2) * DH], F32)
            nc.sync.dma_start(in_b[:], ktv_stage[:])
            nc.gpsimd.collective_compute(
                "AllReduce",
                mybir.AluOpType.add,
                replica_groups=[[0, 1, 2, 3], [4, 5, 6, 7]],
                ins=[in_b.opt()],
                outs=[out_b.opt()],
            )
            ktv_sum = sb.tile([P, (H // 2) * DH], F32, tag="ktv_sum")
            nc.sync.dma_start(ktv_sum[:], out_b[:])
            ktv_bf = sb.tile([P, (H // 2) * DH], BF16, tag="ktv_bf")
            nc.vector.tensor_copy(ktv_bf[:], ktv_sum[:])

            # ---- q projection (transposed layout [e_out, tokens]), overlaps AR
            wq_sb = []
            for c in range(KC):
                t_ = sb.tile([P, E], BF16, tag=f"wq{c}")
                nc.sync.dma_start(t_[:], wqt[c * P:(c + 1) * P, :])
                wq_sb.append(t_)
            bq_sb = sb.tile([P, KC], F32, tag="bq")
            nc.sync.dma_start(bq_sb[:], bq64.ap().rearrange("(m p) -> p m", p=P))
            q_sb = [sb.tile([P, TPC], BF16, tag=f"q{m}", name=f"q{m}") for m in range(KC)]
            for m in range(KC):
                ps = psA.tile([P, TPC], F32, tag="psA")
                for c in range(KC):
                    nc.tensor.matmul(
                        ps[:],
                        wq_sb[c][:, m * P:(m + 1) * P],
                        hs_sb[c][:, :],
                        start=(c == 0),
                        stop=(c == KC - 1),
                    )
                # q epilogue folds bias and the net 1/64 attention scaling
                nc.scalar.activation(
                    q_sb[m][:], ps[:], Ident, bias=bq_sb[:, m:m + 1], scale=1.0 / 64.0
                )

            # ---- O^T per head: OT_h[dv, t] = lhsT(KTV_h).T @ qT_h
            # head pair shares a PSUM tile; odd head uses partitions 64-127
            # end-to-end (lhsT, rhs, out, copy) so base partitions line up.
            oT_sb = [sb.tile([P, TPC], BF16, tag=f"oT{m}", name=f"oT{m}") for m in range(KC)]
            for j in range(H // 2):
                ps = psC.tile([P, TPC], F32, tag="psC")
                for hh in range(2):
                    r0 = hh * DH
                    nc.tensor.matmul(
                        ps[r0:r0 + DH, :],
                        ktv_bf[r0:r0 + DH, j * DH:(j + 1) * DH],
                        q_sb[j][r0:r0 + DH, :],
                        start=True,
                        stop=True,
                    )
                nc.vector.tensor_copy(oT_sb[j][:, :], ps[:])

            # ---- output projection (transposed layout) + bias + DMA out
            wo_sb = []
            for c in range(KC):
                t_ = sb.tile([P, E], BF16, tag=f"wo{c}")
                nc.sync.dma_start(t_[:], wot[c * P:(c + 1) * P, :])
                wo_sb.append(t_)
            bo_sb = sb.tile([P, KC], F32, tag="bo")
            nc.sync.dma_start(bo_sb[:], bo_d.ap().rearrange("(m p) -> p m", p=P))
            for m in range(KC):
                ps = psA.tile([P, TPC], F32, tag="psA")
                for c in range(KC):
                    nc.tensor.matmul(
                        ps[:],
                        wo_sb[c][:, m * P:(m + 1) * P],
                        oT_sb[c][:, :],
                        start=(c == 0),
                        stop=(c == KC - 1),
                    )
                o_f32 = stg.tile([P, TPC], F32, tag="ostg")
                nc.scalar.activation(
                    o_f32[:], ps[:], Ident, bias=bo_sb[:, m:m + 1], scale=1.0
                )
                nc.sync.dma_start(outT[m * P:(m + 1) * P, :], o_f32[:])

    nc.compile()
    return nc


_NC_CACHE = None


def _get_nc():
    global _NC_CACHE
    if _NC_CACHE is None:
        _install_axon_profile_hook()
        _NC_CACHE = build()
    return _NC_CACHE


def make_in_maps(hidden_states, Wq, bq, Wk, bk, Wv, bv, Wo, bo):
    f32 = np.float32
    hs_flat = np.asarray(hidden_states, f32).reshape(NTOK, E)
    shared = {
        "wqt": np.ascontiguousarray(np.asarray(Wq, f32).T).astype(NPBF16),
        "wkt": np.ascontiguousarray(np.asarray(Wk, f32).T).astype(NPBF16),
        "wvt": np.ascontiguousarray(np.asarray(Wv, f32).T).astype(NPBF16),
        "wot": np.ascontiguousarray(np.asarray(Wo, f32).T).astype(NPBF16),
        "bq64": (np.asarray(bq, f32) / 64.0).astype(f32),
        "bo": np.asarray(bo, f32),
        "bkb": np.ascontiguousarray(np.broadcast_to(np.asarray(bk, f32), (P, E))),
        "bvb": np.ascontiguousarray(np.broadcast_to(np.asarray(bv, f32), (P, E))),
    }
    in_maps = []
    for i in range(NC):
        hsT_i = np.ascontiguousarray(
            hs_flat[i * TPC:(i + 1) * TPC].T
        ).astype(NPBF16)
        in_maps.append({"hsT": hsT_i, **shared})
    return in_maps


def run(inputs, trace=False, **kw):
    """Run on 8 NeuronCores; returns (full_output [B,T,E] f32, BassKernelResults)."""
    nc = _get_nc()
    in_maps = make_in_maps(**inputs)
    res = run_bass_kernel_spmd(nc, in_maps, list(range(NC)), trace=trace, **kw)
    out_flat = np.empty((NTOK, E), np.float32)
    for i in range(NC):
        out_flat[i * TPC:(i + 1) * TPC] = np.asarray(res.results[i]["outT"]).T
    return out_flat.reshape(B, T, E), res


def kernel(**inputs):
    out, _ = run(inputs, trace=False)
    return out
